# revision 22
# baseline (speedup 1.0000x reference)
"""Trainium2 Bass kernel for WOQ(int4,group=128) Linear -> +add1+add2 -> WOQ Linear -> mul.

Strategy: data-parallel over the 4096 tokens (8 cores x 512 tokens). Each core
dequantizes the full int4 weight matrix on the fly (twice, once per linear) and
runs both matmuls locally; no collectives.

Layouts: all activations live feature-major ([feature, token]) so both matmuls
contract along partitions without any on-chip transpose. Host pre-transposes
inp/add1/add2 and un-transposes the output.

N-axis permutation pi (nibble-major): TRN2 bitwise ops exist only on DVE for
int32, so the unpack runs as (qweight >> 4j) & 0xF over whole int32 row-tiles,
emitting nibble j of all 512 packed columns contiguously. Position
pos = 1024*c' + x maps to n = 8*(x mod 512) + 2*c' + (x >= 512). Everything
n-indexed (scales, zero-points, bias, add1/add2, output, and the second
matmul's contraction order) is kept in pi order; the host permutes in/out.

Zero-points & bias: handled exactly via an extra accumulated matmul per output
tile: psum += C.T @ R where C = [-z*s; bias] (f32r) and R = [group-sums of the
activations; ones] (f32r), so the weight-side dequant only multiplies by s.
Engines: DVE does unpack + scale-mult + epilogue; ACT does the i32->bf16 cast.
"""

import numpy as np
import ml_dtypes

import concourse.bass as bass
from concourse import bacc
import concourse.tile as tile
import concourse.mybir as mybir
from concourse.alu_op_type import AluOpType
from contextlib import ExitStack

BF16 = mybir.dt.bfloat16
F32 = mybir.dt.float32
F32R = mybir.dt.float32r
I32 = mybir.dt.int32

D = 4096          # K == N == 4096
GS = 128          # quant group size
T_CORE = 512      # tokens per core
N_CORES = 8
CHUNK = 1024      # n-positions per psum round (8 banks x 128)


def make_pi(d=D, chunk=CHUNK):
    """position -> natural n column (nibble-major within int32 row-tiles)."""
    pos = np.arange(d, dtype=np.int64)
    cp = pos // chunk            # chunk index
    x = pos % chunk
    j = 2 * cp + (x >= chunk // 2)
    c = x % (chunk // 2)
    return 8 * c + j


def build_program(d=D, t=T_CORE, chunk=CHUNK, cast_eng="scalar", mult_eng="gpsimd", sb_mode="gpsimd"):
    """One-core SPMD program."""
    g_n = d // GS                 # quant groups (32)
    kt_n = d // 128               # contraction tiles (32)
    nch = d // chunk              # chunks (4)
    jt = chunk // 128             # n-tiles per chunk (8)
    npk = d // 8                  # packed int32 cols (512)
    hc = chunk // 2               # columns per nibble-op (512)

    nc = bacc.Bacc()
    qw_d = nc.dram_tensor("qweight", [d, npk], I32, kind="ExternalInput")
    xt_d = nc.dram_tensor("xt_bf", [d, t], BF16, kind="ExternalInput")
    spi_d = nc.dram_tensor("s_pi", [g_n, d], BF16, kind="ExternalInput")
    c_d = nc.dram_tensor("c_mat", [g_n + 1, d], F32R, kind="ExternalInput")
    r1_d = nc.dram_tensor("r1", [g_n + 1, t], F32, kind="ExternalInput")
    e2_d = nc.dram_tensor("e2", [d, g_n + 1], BF16, kind="ExternalInput")
    a12_d = nc.dram_tensor("a12t", [d, t], F32, kind="ExternalInput")
    a1_d = nc.dram_tensor("a1t", [d, t], F32, kind="ExternalInput")
    out_d = nc.dram_tensor("outt", [d, t], F32, kind="ExternalOutput")
    arf_d = nc.dram_tensor("arf_spill", [d, t], F32)

    ec = g_n + 1  # 33

    with tile.TileContext(nc) as tc, ExitStack() as ctx:
        const = ctx.enter_context(tc.tile_pool(name="const", bufs=1))
        resid = ctx.enter_context(tc.tile_pool(name="resid", bufs=1))
        qwp = ctx.enter_context(tc.tile_pool(name="qwp", bufs=1))
        sbp = ctx.enter_context(tc.tile_pool(name="sbp", bufs=6))
        nibp = ctx.enter_context(tc.tile_pool(name="nibp", bufs=4))
        nbfp = ctx.enter_context(tc.tile_pool(name="nbfp", bufs=4))
        wp = ctx.enter_context(tc.tile_pool(name="wp", bufs=4))
        avp = ctx.enter_context(tc.tile_pool(name="avp", bufs=2))
        outp = ctx.enter_context(tc.tile_pool(name="outp", bufs=2))
        rp = ctx.enter_context(tc.tile_pool(name="rp", bufs=2))
        psp = ctx.enter_context(tc.tile_pool(name="psp", bufs=8, space="PSUM"))

        # ---- constants ----
        e2_sb = const.tile([128, kt_n * ec], BF16)
        nc.sync.dma_start(e2_sb[:].rearrange("p (g e) -> p g e", e=ec),
                          e2_d[:].rearrange("(g p) e -> p g e", p=128))

        # ---- residents ----
        xt_sb = resid.tile([128, kt_n * t], BF16)
        for g in range(kt_n):
            nc.sync.dma_start(xt_sb[:, g * t:(g + 1) * t],
                              xt_d[g * 128:(g + 1) * 128, :])
        ar_b = resid.tile([128, kt_n * t], BF16)    # add_res, bf16 (pass2 rhs)

        def rhs1(g):
            return xt_sb[:, g * t:(g + 1) * t]

        def rhs2(g):
            return ar_b[:, g * t:(g + 1) * t]

        for layer in (1, 2):
            rhs = rhs1 if layer == 1 else rhs2

            # R = [group-sums of rhs; ones]: host-supplied for layer 1,
            # E2-matmul for layer 2 (rhs only exists on device).
            r_sb = rp.tile([ec, t], F32R, tag="rr", name=f"r_sb_{layer}")
            if layer == 1:
                r_f = rp.tile([ec, t], F32, tag="rf")
                nc.sync.dma_start(r_f[:], r1_d[:])
                nc.vector.tensor_copy(r_sb[:], r_f[:])
            else:
                ps_a = psp.tile([128, t], F32, tag="ps", name="ps_a_2")[0:ec, :]
                for g in range(kt_n):
                    nc.tensor.matmul(ps_a[:], e2_sb[:, g * ec:(g + 1) * ec],
                                     rhs(g), start=(g == 0), stop=(g == kt_n - 1))
                r_f = rp.tile([ec, t], F32, tag="rf")
                nc.vector.memset(r_f[:], 1.0)
                nc.vector.tensor_copy(r_f[0:ec - 1, :], ps_a[0:ec - 1, :])
                nc.vector.tensor_copy(r_sb[:], r_f[:])

            # resident packed weights for this pass (loaded once)
            qw_res = qwp.tile([128, kt_n * npk], I32, tag="qwres",
                              name=f"qw_res_{layer}")
            for g in range(kt_n):
                if layer == 1:
                    nc.sync.dma_start(qw_res[:, g * npk:(g + 1) * npk],
                                      qw_d[g * 128:(g + 1) * 128, :])
                else:
                    cc, r_i = g // jt, g % jt
                    j_nib = 2 * cc + (1 if r_i >= jt // 2 else 0)
                    c0 = 128 * (r_i % (jt // 2))
                    n0 = 8 * c0 + j_nib
                    nc.sync.dma_start(qw_res[:, g * npk:(g + 1) * npk],
                                      qw_d[n0:n0 + 8 * 127 + 1:8, :])

            for c in range(nch):
                ps = [psp.tile([128, t], F32, tag="ps",
                               name=f"ps_{layer}_{c}_{j}") for j in range(jt)]
                sb_cache = {}
                for g in range(kt_n):
                    qw_t = qw_res[:, g * npk:(g + 1) * npk]
                    if layer == 2:
                        cc, r_i = g // jt, g % jt
                        key = r_i % (jt // 2 if jt >= 2 else 1) % 4
                        m0 = 8 * key
                        s_b = sb_cache.get(key)
                        if s_b is None:
                            s_b = sbp.tile([128, chunk], BF16, tag="sb",
                                           name=f"s_b2_{layer}_{c}_{key}")
                            nc.sync.dma_start(
                                s_b[:],
                                spi_d[m0:m0 + 8, c * chunk:(c + 1) * chunk]
                                .unsqueeze(1).broadcast_to([8, 16, chunk]))
                            sb_cache[key] = s_b
                    else:
                        s_b = sbp.tile([128, chunk], BF16, tag="sb",
                                       name=f"s_b_{layer}_{c}_{g}")
                    if layer == 1:
                        nc.sync.dma_start(
                            s_b[:],
                            spi_d[g:g + 1, c * chunk:(c + 1) * chunk]
                            .partition_broadcast(128))
                    # -- unpack nibbles 2c, 2c+1 of the row-tile (i32) --
                    nib = nibp.tile([128, chunk], I32)
                    nc.vector.tensor_scalar(
                        nib[:, 0:hc], qw_t, 4 * (2 * c), 0xF,
                        AluOpType.logical_shift_right, AluOpType.bitwise_and)
                    nc.vector.tensor_scalar(
                        nib[:, hc:chunk], qw_t, 4 * (2 * c + 1), 0xF,
                        AluOpType.logical_shift_right, AluOpType.bitwise_and)
                    # -- cast (ACT) + scale (DVE) --
                    nbf = nbfp.tile([128, chunk], BF16)
                    if cast_eng == "scalar":
                        nc.scalar.copy(nbf[:], nib[:])
                    else:
                        nc.vector.tensor_copy(nbf[:], nib[:])
                    w_t = wp.tile([128, chunk], BF16)
                    nc.vector.tensor_tensor(w_t[:], nbf[:], s_b[:],
                                            AluOpType.mult)
                    # -- matmuls --
                    for j in range(jt):
                        nc.tensor.matmul(ps[j][:], w_t[:, j * 128:(j + 1) * 128],
                                         rhs(g), start=(g == 0), stop=False)
                # zero-point + bias correction, closes the accumulation
                c_sb = rp.tile([ec, chunk], F32R, tag="cmat",
                               name=f"c_sb_{layer}_{c}")
                nc.sync.dma_start(
                    c_sb[:], c_d[:, c * chunk:(c + 1) * chunk])
                for j in range(jt):
                    nc.tensor.matmul(ps[j][:], c_sb[:, j * 128:(j + 1) * 128],
                                     r_sb[:], start=False, stop=True)
                # epilogue
                for j in range(jt):
                    b_idx = c * jt + j
                    sl = slice(b_idx * t, (b_idx + 1) * t)
                    if layer == 1:
                        a12 = avp.tile([128, t], F32, tag="av")
                        nc.sync.dma_start(
                            a12[:], a12_d[b_idx * 128:(b_idx + 1) * 128, :])
                        ar_t = outp.tile([128, t], F32, tag="out")
                        nc.vector.tensor_tensor(ar_t[:], ps[j][:], a12[:],
                                                AluOpType.add)
                        nc.scalar.copy(ar_b[:, sl], ar_t[:])
                        nc.sync.dma_start(
                            arf_d[b_idx * 128:(b_idx + 1) * 128, :], ar_t[:])
                    else:
                        a1 = avp.tile([128, t], F32, tag="av")
                        nc.sync.dma_start(
                            a1[:], a1_d[b_idx * 128:(b_idx + 1) * 128, :])
                        arf_t = avp.tile([128, t], F32, tag="arf")
                        nc.sync.dma_start(
                            arf_t[:], arf_d[b_idx * 128:(b_idx + 1) * 128, :])
                        y1 = outp.tile([128, t], F32, tag="out")
                        nc.vector.tensor_tensor(y1[:], ps[j][:], a1[:],
                                                AluOpType.add)
                        ot = outp.tile([128, t], F32, tag="out")
                        getattr(nc, mult_eng).tensor_tensor(
                            ot[:], y1[:], arf_t[:], AluOpType.mult)
                        nc.sync.dma_start(
                            out_d[b_idx * 128:(b_idx + 1) * 128, :], ot[:])
    nc.compile()
    return nc


def host_prep(inp, qweight, woq_scales, woq_qzeros, woq_bias, add1, add2,
              d=D, t=T_CORE, n_cores=N_CORES, chunk=CHUNK):
    """Build per-core input maps."""
    g_n = d // GS
    pi = make_pi(d, chunk)
    shifts = (np.arange(8, dtype=np.int32) * 4)
    z = ((woq_qzeros[:, :, None] >> shifts) & 0xF).reshape(g_n, d).astype(np.float32)
    s_pi = woq_scales[:, pi].astype(ml_dtypes.bfloat16)
    zs = (z * woq_scales).astype(np.float32)
    c_mat = np.empty((g_n + 1, d), dtype=np.float32)
    c_mat[:g_n] = -zs[:, pi]
    c_mat[g_n] = woq_bias[pi]
    ec = g_n + 1
    e2 = np.zeros((d, ec), dtype=ml_dtypes.bfloat16)
    e2[np.arange(d), pi // GS] = 1

    x = inp.reshape(-1, d)
    a1 = add1.reshape(-1, d)
    a2 = add2.reshape(-1, d)
    a12 = a1 + a2

    in_maps = []
    for i in range(n_cores):
        sl = slice(i * t, (i + 1) * t)
        xtb = np.ascontiguousarray(x[sl].T).astype(ml_dtypes.bfloat16)
        r1 = np.ones((ec, t), dtype=np.float32)
        r1[:g_n] = xtb.astype(np.float32).reshape(g_n, GS, t).sum(axis=1)
        in_maps.append({
            "qweight": np.ascontiguousarray(qweight),
            "xt_bf": xtb,
            "s_pi": s_pi,
            "c_mat": c_mat,
            "r1": r1,
            "e2": e2,
            "a12t": np.ascontiguousarray(a12[sl][:, pi].T),
            "a1t": np.ascontiguousarray(a1[sl][:, pi].T),
        })
    return in_maps, pi


_CACHE = {}


def kernel(inp, qweight, woq_scales, woq_qzeros, woq_bias, add1, add2,
           group_size=GS, _trace=False, _repeat=1):
    from concourse import bass_utils
    inp = np.asarray(inp, dtype=np.float32)
    qweight = np.asarray(qweight, dtype=np.int32)
    woq_scales = np.asarray(woq_scales, dtype=np.float32)
    woq_qzeros = np.asarray(woq_qzeros, dtype=np.int32)
    woq_bias = np.asarray(woq_bias, dtype=np.float32)
    add1 = np.asarray(add1, dtype=np.float32)
    add2 = np.asarray(add2, dtype=np.float32)

    if "nc" not in _CACHE:
        _CACHE["nc"] = build_program()
    nc = _CACHE["nc"]
    in_maps, pi = host_prep(inp, qweight, woq_scales, woq_qzeros, woq_bias,
                            add1, add2)
    import time as _time
    times = []
    res = None
    for _ in range(max(1, _repeat)):
        t0 = _time.time()
        res = bass_utils.run_bass_kernel_spmd(
            nc, in_maps, list(range(N_CORES)), trace=_trace)
        times.append(_time.time() - t0)
    _CACHE["times"] = times
    out = np.empty((N_CORES * T_CORE, D), dtype=np.float32)
    for i in range(N_CORES):
        outt = res.results[i]["outt"]          # [D(pi), T]
        out[i * T_CORE:(i + 1) * T_CORE][:, pi] = outt.T
    _CACHE["last_result"] = res
    return out.reshape(inp.shape[0], inp.shape[1], D)
